# revision 1
# baseline (speedup 1.0000x reference)
"""Trainium2 Bass kernel for nn_Attention_1503238553757 (LSA attention).

Reference computation (per batch element):
    qkv = x @ w_qkv; q,k,v heads of dim 64
    dots = (q @ k^T) * scale[h]; diagonal masked to -inf
    attn = softmax(dots); out = attn @ v
    y = concat_heads(out) @ w_out + b_out

Sharding: data-parallel over batch (16 batches -> 2 per core x 8 cores).

Per-core plan (projections/scores in fp16 = full-rate, same 10-bit
mantissa as fp32r but 2-byte so LDWEIGHTS gets the fast-weight-load path;
attn@V operands bf16 because exp values overflow fp16's range; all PSUM
accumulation fp32):
  - x [1024, 512] loaded token-major, transposed on PE -> xT [512, 1024]
  - qT,kT channel-major via lhsT=w_qkv, rhs=xT    (scoresT needs ch-major)
  - v token-major via lhsT=xT, rhs=w_qkv[:, v]    (attn@V lhsT needs tok-major)
  - scoresT[j, i] = kT_h-slice @ qT_h  (keys on partitions); heads processed
    in pairs occupying PE row-groups 0-63 / 64-127 concurrently
  - expT = exp(scale_h * scoresT) via ACT (PSUM->SBUF), diag zeroed via
    affine_select (LSA self-token mask)
  - attn@V: lhsT = [v_h | ones] (M=65) accumulated over j-tiles ->
    outT[0:64] = unnormalized out^T, outT[64] = softmax denominators
  - normalize: fast reciprocal + DRAM-bounce partition-broadcast + DVE
    multiply, written as oT (inner-channel-major) = lhsT for out projection
  - y = oT.T @ w_out + b_out, token-major, DMA'd out

Emission is software-pipelined to keep the PE dense (HAM warm): attention
for batch b starts after a minimal projection prefix; batch b+1's x-load
and transposes interleave into batch b's attention pairs; batch b's out
projection interleaves into batch b+1's attention.

PSUM: psB ([128,1024] = 2 banks) x2 + psO ([65,1024] = 2 banks) x2 = 8 banks.
"""

import os
import sys

for _p in ("/opt/trn_rl_repo", "/root/.axon_site/_ro/trn_rl_repo"):
    if os.path.isdir(_p) and _p not in sys.path:
        sys.path.insert(0, _p)

import numpy as np

import concourse.bass as bass
import concourse.bacc as bacc
import concourse.tile as tile
import concourse.mybir as mybir
from concourse.bass_utils import run_bass_kernel_spmd

# Problem constants (hardcoded per harness contract)
B, N, D = 16, 1024, 512
HEADS, DH = 8, 64
N_CORES = 8
BPC = B // N_CORES  # batches per core = 2

dt = mybir.dt
F32 = dt.float32
F32R = dt.float32r
BF16 = dt.bfloat16
F16 = dt.float16
ATT_DT = BF16  # attention-path dtype (scores/attnV operands)
EXP = mybir.ActivationFunctionType.Exp

NT = N // 128   # token tiles = 8
VW = DH + 1     # per-head v width (v | ones)
KD = D // 128   # d/inner k-tiles = 4


def build_program():
    nc = bacc.Bacc("TRN2", target_bir_lowering=False, debug=False,
                   num_devices=N_CORES)

    x = nc.dram_tensor("x", [BPC, N, D], F32, kind="ExternalInput").ap()
    w_qkv = nc.dram_tensor("w_qkv", [D, 3 * D], F32, kind="ExternalInput").ap()
    w_out = nc.dram_tensor("w_out", [D, D], F32, kind="ExternalInput").ap()
    b_out = nc.dram_tensor("b_out", [D], F32, kind="ExternalInput").ap()
    scale = nc.dram_tensor("scale", [HEADS], F32, kind="ExternalInput").ap()
    y = nc.dram_tensor("y", [BPC, N, D], F32, kind="ExternalOutput").ap()

    ident_dram = nc.inline_tensor(np.eye(128, dtype=np.float16), name="ident")
    ones_dram = nc.inline_tensor(np.ones((128, 64), dtype=np.float32), name="ones128")

    import contextlib
    with tile.TileContext(nc) as tc, contextlib.ExitStack() as ctx:
        consts = ctx.enter_context(tc.tile_pool(name="consts", bufs=1))
        p_x = ctx.enter_context(tc.tile_pool(name="p_x", bufs=1))
        p_exp = ctx.enter_context(tc.tile_pool(name="p_exp", bufs=6))
        p_mid = ctx.enter_context(tc.tile_pool(name="p_mid", bufs=3))
        p_qk = ctx.enter_context(tc.tile_pool(name="p_qk", bufs=2))
        p_v = ctx.enter_context(tc.tile_pool(name="p_v", bufs=2))
        p_y = ctx.enter_context(tc.tile_pool(name="p_y", bufs=2))
        p_rb = ctx.enter_context(tc.tile_pool(name="p_rb", bufs=2))
        p_otmp = ctx.enter_context(tc.tile_pool(name="p_otmp", bufs=3))
        p_small = ctx.enter_context(tc.tile_pool(name="p_small", bufs=2))
        psB = ctx.enter_context(tc.tile_pool(name="psB", bufs=2, space="PSUM"))
        psO = ctx.enter_context(tc.tile_pool(name="psO", bufs=2, space="PSUM"))
        p_dram = ctx.enter_context(tc.tile_pool(name="p_dram", bufs=2, space="DRAM"))

        # ---- constants (ident first: transposes only need x + ident) ----
        ident_sb = consts.tile([128, 128], F16)
        nc.sync.dma_start(out=ident_sb, in_=ident_dram.ap())
        wqkv_sb = consts.tile([128, KD, 3 * D], F16)
        wout_sb = consts.tile([128, KD, D], F16)
        bout_bc = consts.tile([128, D], F32)
        scale_sb = consts.tile([128, HEADS], F32)

        def emit_const_loads():
            nc.gpsimd.dma_start(
                out=wqkv_sb,
                in_=w_qkv.rearrange("(k p) c -> p k c", p=128),
            )
            nc.gpsimd.dma_start(
                out=wout_sb,
                in_=w_out.rearrange("(k p) c -> p k c", p=128),
            )
            nc.gpsimd.dma_start(
                out=bout_bc,
                in_=bass.AP(tensor=b_out.tensor, offset=0,
                            ap=[[0, 128], [1, D]]),
            )
            nc.gpsimd.dma_start(
                out=scale_sb,
                in_=bass.AP(tensor=scale.tensor, offset=0,
                            ap=[[0, 128], [1, HEADS]]),
            )

        # per-batch state kept across the pipelined emission
        xT = [None] * BPC
        qkT = [None] * BPC
        vsb = [None] * BPC
        osb = [None] * BPC

        def emit_load_x(b):
            x_sb = p_x.tile([128, NT, D], F16, tag="x", name=f"x_sb{b}")
            src = x[b].rearrange("(r p) d -> p r d", p=128)
            nc.gpsimd.dma_start(out=x_sb[:, 0:NT // 2, :], in_=src[:, 0:NT // 2, :])
            nc.gpsimd.dma_start(out=x_sb[:, NT // 2:, :], in_=src[:, NT // 2:, :])
            return x_sb

        def emit_transpose_half(b, x_sb, kd, half):
            ps_t = psB.tile([128, 512], F16, tag="psB",
                            name=f"ps_t_{b}_{kd}_{half}")
            for rr in range(4):
                r = 4 * half + rr
                nc.tensor.transpose(
                    ps_t[:, 128 * rr:128 * rr + 128],
                    x_sb[:, r, 128 * kd:128 * kd + 128],
                    ident_sb,
                )
            nc.vector.tensor_copy(
                xT[b][:, kd, 512 * half:512 * half + 512], ps_t
            )

        def emit_transposes(b, x_sb, kds):
            for half in range(2):
                for kd in kds:
                    emit_transpose_half(b, x_sb, kd, half)

        def emit_qk_half(b, ct, nh):
            ps_qk = psB.tile([128, 512], F32, tag="psB",
                             name=f"ps_qk_{b}_{ct}_{nh}")
            for kt in range(KD):
                nc.tensor.matmul(
                    ps_qk,
                    wqkv_sb[:, kt, 128 * ct:128 * ct + 128],
                    xT[b][:, kt, 512 * nh:512 * nh + 512],
                    start=(kt == 0), stop=(kt == KD - 1),
                )
            nc.vector.tensor_copy(
                qkT[b][:, ct, 512 * nh:512 * nh + 512], ps_qk
            )

        def emit_qk_ct(b, ct):
            for nh in range(2):
                emit_qk_half(b, ct, nh)

        def emit_v_r(b, r):
            ps_v = psB.tile([128, 512], F32, tag="psB", name=f"ps_v_{b}_{r}")
            for kt in range(KD):
                nc.tensor.matmul(
                    ps_v,
                    xT[b][:, kt, 128 * r:128 * r + 128],
                    wqkv_sb[:, kt, 2 * D:3 * D],
                    start=(kt == 0), stop=(kt == KD - 1),
                )
            nc.vector.tensor_copy(
                vsb[b][:, r, 0:HEADS * VW].rearrange(
                    "p (h e) -> p h e", h=HEADS)[:, :, 0:DH],
                ps_v.rearrange("p (h e) -> p h e", h=HEADS),
            )

        def emit_ones(b):
            nc.vector.memset(
                vsb[b][:, :, 0:HEADS * VW].rearrange(
                    "p r (h e) -> p r h e", h=HEADS)[:, :, :, DH:DH + 1],
                1.0,
            )
            # pad region read as garbage weights by the last head's widened
            # attn@V lhsT; keep it finite
            nc.vector.memset(vsb[b][:, :, HEADS * VW:], 1.0)

        def emit_head_pair(b, g, filler=None, pairs_left=1, last=False):
            """Attention for heads (2g, 2g+1) of batch b; the two heads
            occupy PE row groups 0-63 / 64-127 concurrently.
            filler: queue of PSUM-using chunks, popped adaptively at odd jt.
            light: queue of DVE/DMA-only chunks, popped at even jt."""
            heads = (2 * g, 2 * g + 1)
            ps_os = {h: psO.tile([DH + 1, N], F32, tag="psO", name=f"ps_o_{b}_{h}") for h in heads}
            for jt in range(NT):
                tiles = {
                    h: psB.tile([128, N], F32, tag="psB",
                                name=f"ps_s_{b}_{h}_{jt}")
                    for h in heads
                }
                for ih in range(2):
                    for h in heads:
                        q_off = (h % 2) * 64
                        nc.tensor.matmul(
                            tiles[h][:, 512 * ih:512 * ih + 512],
                            qkT[b][q_off:q_off + 64, 4 + g,
                                   128 * jt:128 * jt + 128],
                            qkT[b][q_off:q_off + 64, g,
                                   512 * ih:512 * ih + 512],
                            start=True, stop=True,
                        )
                for h in heads:
                    expT = p_exp.tile([128, N], ATT_DT, tag="exp")
                    nc.scalar.activation(
                        expT, tiles[h], EXP, scale=scale_sb[:, h:h + 1]
                    )
                    nc.gpsimd.affine_select(
                        out=expT[:, 128 * jt:128 * jt + 128],
                        in_=expT[:, 128 * jt:128 * jt + 128],
                        compare_op=mybir.AluOpType.not_equal,
                        fill=0.0, base=0, channel_multiplier=1,
                        pattern=[[-1, 128]],
                    )
                    for ih in range(2):
                        nc.tensor.matmul(
                            ps_os[h][:, 512 * ih:512 * ih + 512],
                            vsb[b][:, jt, VW * h:VW * h + DH + 1],
                            expT[:, 512 * ih:512 * ih + 512],
                            start=(jt == 0), stop=(jt == NT - 1),
                        )
                if filler and jt % 2 == 1:
                    import math as _math
                    npop = max(1, _math.ceil(len(filler) / (pairs_left * 4)))
                    for _ in range(min(npop, len(filler))):
                        filler.pop(0)()
            if last:
                # tail pair: skip the o_tmp bounce (no later work needs the
                # PSUM slots); interleave both heads' chains so the DMA hops
                # of one head hide under the DVE ops of the other
                sums, recips, rbs = {}, {}, {}
                for h in heads:
                    sums[h] = p_small.tile([1, N], F32, tag="sums",
                                           name=f"sums_{b}_{h}")
                    nc.vector.tensor_copy(sums[h], ps_os[h][DH:DH + 1, :])
                for h in heads:
                    recips[h] = p_small.tile([1, N], F32, tag="recip",
                                             name=f"recip_{b}_{h}")
                    nc.vector.reciprocal_approx_fast(recips[h], sums[h])
                    scr = p_dram.tile([1, N], F32, tag="scr",
                                      name=f"scr_{b}_{h}")
                    nc.gpsimd.dma_start(out=scr, in_=recips[h])
                    rbs[h] = p_rb.tile([64, N], F32, tag="rb",
                                       name=f"rb_{b}_{h}")
                    nc.gpsimd.dma_start(
                        out=rbs[h],
                        in_=bass.AP(tensor=scr.tensor, offset=scr.offset,
                                    ap=[[0, 64], [1, N]]),
                    )
                for h in heads:
                    q_off = (h % 2) * 64
                    nc.vector.tensor_mul(
                        osb[b][q_off:q_off + 64, g, :], ps_os[h][0:DH, :],
                        rbs[h],
                    )
                return
            for h in heads:
                q_off = (h % 2) * 64
                # free the PSUM slot fast: single copy of out^T + sums row
                o_tmp = p_otmp.tile([DH + 1, N], F32, tag="otmp",
                                    name=f"o_tmp_{b}_{h}")
                nc.vector.tensor_copy(o_tmp, ps_os[h])
                sums_sb = p_small.tile([1, N], F32, tag="sums")
                nc.vector.tensor_copy(sums_sb, o_tmp[DH:DH + 1, :])
                recip = p_small.tile([1, N], F32, tag="recip")
                nc.vector.reciprocal_approx_fast(recip, sums_sb)
                scr = p_dram.tile([1, N], F32, tag="scr")
                nc.gpsimd.dma_start(out=scr, in_=recip)
                rb = p_rb.tile([64, N], F32, tag="rb")
                nc.gpsimd.dma_start(
                    out=rb,
                    in_=bass.AP(tensor=scr.tensor, offset=scr.offset,
                                ap=[[0, 64], [1, N]]),
                )
                nc.vector.tensor_mul(
                    osb[b][q_off:q_off + 64, g, :], o_tmp[0:DH, :], rb
                )

        def emit_yproj_r(b, r):
            ps_y = psB.tile([128, 512], F32, tag="psB", name=f"ps_y_{b}_{r}")
            for kt in range(KD):
                nc.tensor.matmul(
                    ps_y,
                    osb[b][:, kt, 128 * r:128 * r + 128],
                    wout_sb[:, kt, :],
                    start=(kt == 0), stop=(kt == KD - 1),
                )
            y_sb = p_y.tile([128, D], F32, tag="y")
            nc.vector.tensor_add(y_sb, ps_y, bout_bc)
            nc.sync.dma_start(
                out=y[b, 128 * r:128 * r + 128, :], in_=y_sb
            )

        # ================= pipelined emission =================
        import functools
        F = functools.partial

        # batch 0 prologue: load + transpose + minimal projection prefix
        x0 = emit_load_x(0)
        emit_const_loads()
        xT[0] = p_mid.tile([128, KD, N], F16, tag="mid", name="xT0")
        qkT[0] = p_qk.tile([128, 8, N], F16, tag="qk", name="qkT0")
        vsb[0] = p_v.tile([128, NT, HEADS * VW + 64], ATT_DT, tag="v", name="v0")
        osb[0] = p_mid.tile([128, KD, N], F16, tag="mid", name="o0")
        emit_transposes(0, x0, range(KD))
        emit_ones(0)
        emit_qk_ct(0, 0)       # q heads 0,1
        emit_qk_ct(0, 4)       # k heads 0,1
        for r in range(NT):
            emit_v_r(0, r)

        # batch 1 x-load can start as soon as x0's slot frees
        x1 = emit_load_x(1)
        xT[1] = p_mid.tile([128, KD, N], F16, tag="mid", name="xT1")

        vsb[1] = p_v.tile([128, NT, HEADS * VW + 64], ATT_DT, tag="v", name="v1")
        osb[1] = p_mid.tile([128, KD, N], F16, tag="mid", name="o1")
        qkT[1] = p_qk.tile([128, 8, N], F16, tag="qk", name="qkT1")

        # C(0) filler queue: remaining B(0) qk chunks + all of A(1)/B(1)
        q0 = []
        for ct in (1, 5, 2, 6, 3, 7):
            for nh in range(2):
                q0.append(F(emit_qk_half, 0, ct, nh))
        for kd in range(KD):
            for half in range(2):
                q0.append(F(emit_transpose_half, 1, x1, kd, half))
        q0.append(F(emit_ones, 1))
        for ct in (0, 4):
            for nh in range(2):
                q0.append(F(emit_qk_half, 1, ct, nh))
        for r in range(NT):
            q0.append(F(emit_v_r, 1, r))

        for g in range(4):
            emit_head_pair(0, g, filler=q0, pairs_left=4 - g)

        while q0:
            q0.pop(0)()

        # C(1) filler queue: remaining B(1) qk chunks + D(0)
        q1 = []
        for ct in (1, 5, 2, 6, 3, 7):
            for nh in range(2):
                q1.append(F(emit_qk_half, 1, ct, nh))
        for r in range(NT):
            q1.append(F(emit_yproj_r, 0, r))

        for g in range(4):
            emit_head_pair(1, g, filler=q1, pairs_left=4 - g, last=(g == 3))
        while q1:
            q1.pop(0)()

        # D(1) tail
        for r in range(NT):
            emit_yproj_r(1, r)

    nc.compile()
    return nc


_NC = None


def _get_program():
    global _NC
    if _NC is None:
        _NC = build_program()
    return _NC


def make_in_maps(x, w_qkv, w_out, b_out, scale):
    x = np.ascontiguousarray(np.asarray(x, dtype=np.float32))
    w_qkv = np.ascontiguousarray(np.asarray(w_qkv, dtype=np.float32))
    w_out = np.ascontiguousarray(np.asarray(w_out, dtype=np.float32))
    b_out = np.ascontiguousarray(np.asarray(b_out, dtype=np.float32))
    scale = np.ascontiguousarray(np.asarray(scale, dtype=np.float32))
    return [
        {
            "x": x[c * BPC:(c + 1) * BPC],
            "w_qkv": w_qkv,
            "w_out": w_out,
            "b_out": b_out,
            "scale": scale,
        }
        for c in range(N_CORES)
    ]


def kernel(x, w_qkv, w_out, b_out, scale):
    nc = _get_program()
    in_maps = make_in_maps(x, w_qkv, w_out, b_out, scale)
    res = run_bass_kernel_spmd(nc, in_maps, core_ids=list(range(N_CORES)))
    return np.concatenate([res.results[c]["y"] for c in range(N_CORES)], axis=0)


if __name__ == "__main__":
    rng = np.random.default_rng(0)
    inputs = {
        "x": rng.standard_normal((B, N, D), dtype=np.float32),
        "w_qkv": rng.standard_normal((D, 3 * D), dtype=np.float32) * 0.03,
        "w_out": rng.standard_normal((D, D), dtype=np.float32) * 0.04,
        "b_out": np.zeros(D, dtype=np.float32),
        "scale": np.full(HEADS, DH ** -0.5, dtype=np.float32),
    }
    out = kernel(**inputs)
    print("kernel output", out.shape, out.dtype)



# revision 2
# speedup vs baseline: 1.1843x; 1.1843x over previous
"""Trainium2 Bass kernel for nn_Attention_1503238553757 (LSA attention).

Reference computation (per batch element):
    qkv = x @ w_qkv; q,k,v heads of dim 64
    dots = (q @ k^T) * scale[h]; diagonal masked to -inf
    attn = softmax(dots); out = attn @ v
    y = concat_heads(out) @ w_out + b_out

Sharding: data-parallel over batch (16 batches -> 2 per core x 8 cores).

Per-core schedule (v2 — engine-rebalanced, round-pipelined):
  - scores head pairs emitted adjacently -> PE row-group packing (two K=64
    matmuls run concurrently in row groups 0-63 / 64-127, ~2x score rate)
  - exp is split between the Scalar engine (true exp, per-head scale via
    activation scale AP) and the Vector engine (Schraudolph bit-trick exp:
    bf16 = bitcast(int16(round(A*scale*x + B))), max rel err ~4%, washed
    out by softmax renormalization + diffuse attention averaging)
  - diagonal self-token mask: affine_select on GpSimd (SBUF only)
  - attn@V with (v | ones) stationary -> out^T rows + denominator row in
    PSUM; evacuated to SBUF by ACT/DVE (load-balanced)
  - denominator reciprocal: DMA bounce spreads the [1,N] row to [128,8]
    so reciprocal_approx_fast costs ~8 cycles, then a second bounce
    broadcasts 1/denom to [64,N]; normalize multiply runs on GpSimd
    (all-SBUF), writing osb = yproj lhsT in f16
  - projections (qkv, v, x-transposes, y-proj) are deadline-scheduled
    filler units riding the scores PSUM ring between attention rounds
  - emission is round-based: scores(r) | exp(r) | selects(r) | filler |
    attnV(r-1), so every engine queue follows round order
"""

import os
import sys

for _p in ("/opt/trn_rl_repo", "/root/.axon_site/_ro/trn_rl_repo"):
    if os.path.isdir(_p) and _p not in sys.path:
        sys.path.insert(0, _p)

import numpy as np

import concourse.bass as bass
import concourse.bacc as bacc
import concourse.tile as tile
import concourse.mybir as mybir
from concourse.bass_utils import run_bass_kernel_spmd

# Problem constants (hardcoded per harness contract)
B, N, D = 16, 1024, 512
HEADS, DH = 8, 64
N_CORES = 8
BPC = B // N_CORES  # batches per core = 2

dt = mybir.dt
F32 = dt.float32
BF16 = dt.bfloat16
F16 = dt.float16
I16 = dt.int16
EXP = mybir.ActivationFunctionType.Exp
MUL = mybir.AluOpType.mult
ADD = mybir.AluOpType.add

NT = N // 128   # token tiles = 8
VW = DH + 1     # per-head v width (v | ones)
KD = D // 128   # d/inner k-tiles = 4

# Schraudolph bf16-exp constants (DVE f32->int16 is round-to-nearest,
# verified on HW): exp(x) ~= bitcast_bf16(int16(A16*x + B16))
A16 = 128.0 / float(np.log(2.0))     # 184.6650
B16 = 127.0 * 128.0 - 7.4115         # 16248.59


class EngBal:
    """Static load balancer between the Scalar (act) and Vector (dve)
    engines for PSUM-consuming ops."""

    def __init__(self, nc):
        self.nc = nc
        self.t = {"act": 0.0, "dve": 0.0}

    def pick(self, cost_act, cost_dve):
        if self.t["act"] + cost_act <= self.t["dve"] + cost_dve:
            self.t["act"] += cost_act
            return "act"
        self.t["dve"] += cost_dve
        return "dve"

    def add(self, eng, cost):
        self.t[eng] += cost


def build_program():
    nc = bacc.Bacc("TRN2", target_bir_lowering=False, debug=False,
                   num_devices=N_CORES)

    x = nc.dram_tensor("x", [BPC, N, D], F32, kind="ExternalInput").ap()
    w_qkv = nc.dram_tensor("w_qkv", [D, 3 * D], F32, kind="ExternalInput").ap()
    w_out = nc.dram_tensor("w_out", [D, D], F32, kind="ExternalInput").ap()
    b_out = nc.dram_tensor("b_out", [D], F32, kind="ExternalInput").ap()
    scale = nc.dram_tensor("scale", [HEADS], F32, kind="ExternalInput").ap()
    y = nc.dram_tensor("y", [BPC, N, D], F32, kind="ExternalOutput").ap()

    ident_dram = nc.inline_tensor(np.eye(128, dtype=np.float16), name="ident")

    bal = EngBal(nc)

    import contextlib
    with tile.TileContext(nc) as tc, contextlib.ExitStack() as ctx:
        consts = ctx.enter_context(tc.tile_pool(name="consts", bufs=1))
        p_x = ctx.enter_context(tc.tile_pool(name="p_x", bufs=1))
        p_big = ctx.enter_context(tc.tile_pool(name="p_big", bufs=2))
        p_exp = ctx.enter_context(tc.tile_pool(name="p_exp", bufs=6))
        p_on = ctx.enter_context(tc.tile_pool(name="p_on", bufs=4))
        p_rb = ctx.enter_context(tc.tile_pool(name="p_rb", bufs=4))
        p_sm = ctx.enter_context(tc.tile_pool(name="p_sm", bufs=4))
        p_y = ctx.enter_context(tc.tile_pool(name="p_y", bufs=3))
        psS = ctx.enter_context(tc.tile_pool(name="psS", bufs=2, space="PSUM"))
        psO = ctx.enter_context(tc.tile_pool(name="psO", bufs=2, space="PSUM"))
        p_dram = ctx.enter_context(tc.tile_pool(name="p_dram", bufs=4,
                                                space="DRAM"))

        # ---------------- constants ----------------
        ident_sb = consts.tile([128, 128], F16)
        nc.sync.dma_start(out=ident_sb, in_=ident_dram.ap())
        wqkv_sb = consts.tile([128, KD, 3 * D], F16)
        wout_sb = consts.tile([128, KD, D], F16)
        bout_bc = consts.tile([128, D], F32)
        scale_sb = consts.tile([128, HEADS], F32)
        scale_schr = consts.tile([128, HEADS], F32)

        def emit_const_dmas_early():
            # q/k columns of w_qkv first (prologue critical path)
            nc.gpsimd.dma_start(
                out=wqkv_sb[:, :, 0:2 * D],
                in_=w_qkv.rearrange("(k p) c -> p k c", p=128)[:, :, 0:2 * D],
            )
            nc.gpsimd.dma_start(
                out=wqkv_sb[:, :, 2 * D:3 * D],
                in_=w_qkv.rearrange("(k p) c -> p k c", p=128)[:, :, 2 * D:3 * D],
            )
            nc.sync.dma_start(
                out=bout_bc,
                in_=bass.AP(tensor=b_out.tensor, offset=0,
                            ap=[[0, 128], [1, D]]),
            )
            nc.sync.dma_start(
                out=scale_sb,
                in_=bass.AP(tensor=scale.tensor, offset=0,
                            ap=[[0, 128], [1, HEADS]]),
            )
            nc.vector.tensor_scalar_mul(scale_schr, scale_sb, float(A16))

        def emit_const_dmas_late():
            nc.gpsimd.dma_start(
                out=wout_sb,
                in_=w_out.rearrange("(k p) c -> p k c", p=128),
            )

        # ---------------- per-batch state ----------------
        xT = [None] * BPC
        qkT = [None] * BPC
        vsb = [None] * BPC
        osb = [None] * BPC

        def alloc_batch(b):
            xT[b] = p_big.tile([128, KD, N], F16, tag="xT", name=f"xT{b}")
            qkT[b] = p_big.tile([128, 8, N], F16, tag="qk", name=f"qkT{b}")
            vsb[b] = p_big.tile([128, NT, HEADS * VW + 64], BF16, tag="v",
                                name=f"v{b}")
            osb[b] = p_big.tile([128, KD, N], F16, tag="o", name=f"o{b}")

        def emit_load_x(b):
            x_sb = p_x.tile([128, NT, D], F16, tag="x", name=f"x_sb{b}")
            src = x[b].rearrange("(r p) d -> p r d", p=128)
            for c in range(4):
                nc.gpsimd.dma_start(out=x_sb[:, 2 * c:2 * c + 2, :],
                                    in_=src[:, 2 * c:2 * c + 2, :])
            return x_sb

        def emit_ones(b):
            nc.gpsimd.memset(
                vsb[b][:, :, 0:HEADS * VW].rearrange(
                    "p r (h e) -> p r h e", h=HEADS)[:, :, :, DH:DH + 1],
                1.0,
            )
            nc.gpsimd.memset(vsb[b][:, :, HEADS * VW:], 1.0)

        # ---------------- filler units (ride the psS ring) ----------------
        def evac(dst_ap, src_ap, cost_scale=1.0):
            e = bal.pick(0.67 * cost_scale, 0.73 * cost_scale)
            if e == "act":
                nc.scalar.copy(dst_ap, src_ap)
            else:
                nc.vector.tensor_copy(dst_ap, src_ap)

        def u_tr(b, x_sb, kd, half):
            ps_t = psS.tile([128, 512], F16, tag="s", name=f"ps_t{b}_{kd}_{half}")
            for rr in range(4):
                r = 4 * half + rr
                nc.tensor.transpose(
                    ps_t[:, 128 * rr:128 * rr + 128],
                    x_sb[:, r, 128 * kd:128 * kd + 128],
                    ident_sb,
                )
            evac(xT[b][:, kd, 512 * half:512 * half + 512], ps_t)

        def u_qk(b, ct, nh):
            ps_qk = psS.tile([128, 512], F32, tag="s", name=f"ps_qk{b}_{ct}_{nh}")
            for kt in range(KD):
                nc.tensor.matmul(
                    ps_qk,
                    wqkv_sb[:, kt, 128 * ct:128 * ct + 128],
                    xT[b][:, kt, 512 * nh:512 * nh + 512],
                    start=(kt == 0), stop=(kt == KD - 1),
                )
            evac(qkT[b][:, ct, 512 * nh:512 * nh + 512], ps_qk)

        def u_v(b, r):
            ps_v = psS.tile([128, 512], F32, tag="s", name=f"ps_v{b}_{r}")
            for kt in range(KD):
                nc.tensor.matmul(
                    ps_v,
                    xT[b][:, kt, 128 * r:128 * r + 128],
                    wqkv_sb[:, kt, 2 * D:3 * D],
                    start=(kt == 0), stop=(kt == KD - 1),
                )
            evac(
                vsb[b][:, r, 0:HEADS * VW].rearrange(
                    "p (h e) -> p h e", h=HEADS)[:, :, 0:DH],
                ps_v.rearrange("p (h e) -> p h e", h=HEADS),
            )

        def u_yp(b, r):
            ps_y = psS.tile([128, 512], F32, tag="s", name=f"ps_y{b}_{r}")
            for kt in range(KD):
                nc.tensor.matmul(
                    ps_y,
                    osb[b][:, kt, 128 * r:128 * r + 128],
                    wout_sb[:, kt, :],
                    start=(kt == 0), stop=(kt == KD - 1),
                )
            y_sb = p_y.tile([128, D], F32, tag="y")
            nc.vector.tensor_tensor(y_sb, ps_y, bout_bc, op=ADD)
            bal.add("dve", 0.73)
            nc.sync.dma_start(out=y[b, 128 * r:128 * r + 128, :], in_=y_sb)

        # ---------------- attention rounds ----------------
        def emit_scores(b, g, jt):
            heads = (2 * g, 2 * g + 1)
            tiles = {h: psS.tile([128, N], F32, tag="s",
                                 name=f"ps_s{b}_{h}_{jt}") for h in heads}
            for ih in range(2):
                for h in heads:  # adjacent -> row-group packed
                    q_off = (h % 2) * 64
                    nc.tensor.matmul(
                        tiles[h][:, 512 * ih:512 * ih + 512],
                        qkT[b][q_off:q_off + 64, 4 + g, 128 * jt:128 * jt + 128],
                        qkT[b][q_off:q_off + 64, g, 512 * ih:512 * ih + 512],
                        start=True, stop=True,
                    )
            return tiles

        def emit_exp(b, g, jt, s_tiles):
            expts = {}
            for h in sorted(s_tiles):
                expT = p_exp.tile([128, N], BF16, tag="exp",
                                  name=f"expT{b}_{h}_{jt}")
                e = bal.pick(1.10, 1.32)
                if e == "act":
                    nc.scalar.activation(expT, s_tiles[h], EXP,
                                         scale=scale_sb[:, h:h + 1])
                else:
                    nc.vector.tensor_scalar(
                        out=expT.bitcast(I16), in0=s_tiles[h],
                        scalar1=scale_schr[:, h:h + 1], scalar2=float(B16),
                        op0=MUL, op1=ADD,
                    )
                nc.gpsimd.affine_select(
                    out=expT[:, 128 * jt:128 * jt + 128],
                    in_=expT[:, 128 * jt:128 * jt + 128],
                    compare_op=mybir.AluOpType.not_equal,
                    fill=0.0, base=0, channel_multiplier=1,
                    pattern=[[-1, 128]],
                )
                expts[h] = expT
            return expts

        ps_os = {}

        def emit_attnv(b, g, jt, expts):
            heads = (2 * g, 2 * g + 1)
            if jt == 0:
                for h in heads:
                    ps_os[h] = psO.tile([DH + 1, N], F32, tag="o",
                                        name=f"ps_o{b}_{h}")
            for h in heads:
                for ih in range(2):
                    nc.tensor.matmul(
                        ps_os[h][:, 512 * ih:512 * ih + 512],
                        vsb[b][:, jt, VW * h:VW * h + DH + 1],
                        expts[h][:, 512 * ih:512 * ih + 512],
                        start=(jt == 0), stop=(jt == NT - 1),
                    )

        def emit_finish_pair(b, g):
            """After attnV(g, jt=7): evacuate psO, reciprocal via DMA
            bounces, normalize on GpSimd into osb."""
            heads = (2 * g, 2 * g + 1)
            o_un, rbs = {}, {}
            for h in heads:
                o_un[h] = p_on.tile([DH + 1, N], F32, tag="on",
                                    name=f"o_un{b}_{h}")
                evac(o_un[h], ps_os[h], cost_scale=1.6)
            for h in heads:
                scrA = p_dram.tile([1, N], F32, tag="scrA", name=f"scrA{b}_{h}")
                nc.sync.dma_start(out=scrA, in_=o_un[h][DH:DH + 1, :])
                s128 = p_sm.tile([128, N // 128], F32, tag="s128",
                                 name=f"s128_{b}_{h}")
                nc.sync.dma_start(
                    out=s128,
                    in_=bass.AP(tensor=scrA.tensor, offset=scrA.offset,
                                ap=[[N // 128, 128], [1, N // 128]]),
                )
                r128 = p_sm.tile([128, N // 128], F32, tag="r128",
                                 name=f"r128_{b}_{h}")
                nc.vector.reciprocal_approx_fast(r128, s128)
                bal.add("dve", 0.1)
                scrB = p_dram.tile([1, N], F32, tag="scrB", name=f"scrB{b}_{h}")
                nc.sync.dma_start(
                    out=bass.AP(tensor=scrB.tensor, offset=scrB.offset,
                                ap=[[N // 128, 128], [1, N // 128]]),
                    in_=r128,
                )
                rbs[h] = p_rb.tile([64, N], F32, tag="rb", name=f"rb{b}_{h}")
                nc.sync.dma_start(
                    out=rbs[h],
                    in_=bass.AP(tensor=scrB.tensor, offset=scrB.offset,
                                ap=[[0, 64], [1, N]]),
                )
            for h in heads:
                q_off = (h % 2) * 64
                nc.gpsimd.tensor_tensor(
                    osb[b][q_off:q_off + 64, g, :], o_un[h][0:DH, :],
                    rbs[h], op=MUL,
                )

        # ================= emission schedule =================
        import functools
        F = functools.partial

        # ---- prologue ----
        x0 = emit_load_x(0)
        emit_const_dmas_early()
        alloc_batch(0)
        for half in range(2):
            for kd in range(KD):
                u_tr(0, x0, kd, half)
        emit_ones(0)
        for ct in (0, 4):
            for nh in range(2):
                u_qk(0, ct, nh)
        u_v(0, 0)
        u_v(0, 1)

        # ---- filler queue: (deadline_round, emit_fn) ----
        q = []
        for i, r in enumerate(range(2, NT)):          # v(0) r2..7
            q.append((r - 1, F(u_v, 0, r)))
        for p, (ctq, ctk) in enumerate(((1, 5), (2, 6), (3, 7)), start=1):
            dl = 8 * p - 6
            q.append((dl, F(u_qk, 0, ctq, 0)))
            q.append((dl + 1, F(u_qk, 0, ctq, 1)))
            q.append((dl + 2, F(u_qk, 0, ctk, 0)))
            q.append((dl + 3, F(u_qk, 0, ctk, 1)))

        x1 = [None]

        def start_b1_load():
            x1[0] = emit_load_x(1)
            emit_const_dmas_late()
            alloc_batch(1)

        q.append((1, start_b1_load))
        for i, (kd, half) in enumerate(
                [(kd, half) for half in range(2) for kd in range(KD)]):
            q.append((6 + i, F(lambda kd=kd, half=half: u_tr(1, x1[0], kd, half))))
        q.append((14, F(emit_ones, 1)))
        for i, r in enumerate(range(2)):              # v(1) r0,r1 early
            q.append((22 + i, F(u_v, 1, r)))
        for i, (ct, nh) in enumerate([(0, 0), (0, 1), (4, 0), (4, 1)]):
            q.append((25 + i, F(u_qk, 1, ct, nh)))
        for i, r in enumerate(range(2, NT)):          # v(1) r2..7
            q.append((29 + i, F(u_v, 1, r)))
        for p, (ctq, ctk) in enumerate(((1, 5), (2, 6), (3, 7)), start=1):
            dl = 32 + 8 * p - 6
            q.append((dl, F(u_qk, 1, ctq, 0)))
            q.append((dl + 1, F(u_qk, 1, ctq, 1)))
            q.append((dl + 2, F(u_qk, 1, ctk, 0)))
            q.append((dl + 3, F(u_qk, 1, ctk, 1)))
        for i in range(NT):                            # yproj(0)
            q.append((38 + 2 * i, F(u_yp, 0, i)))
        q.sort(key=lambda e: e[0])

        # ---- attention rounds with lag-1 attnV ----
        rounds = [(b, g, jt) for b in range(BPC) for g in range(4)
                  for jt in range(NT)]
        prev = None
        for ridx, (b, g, jt) in enumerate(rounds):
            s_tiles = emit_scores(b, g, jt)
            expts = emit_exp(b, g, jt, s_tiles)
            while q and q[0][0] <= ridx:
                q.pop(0)[1]()
            if prev is not None:
                pb, pg, pjt, pexp = prev
                emit_attnv(pb, pg, pjt, pexp)
                if pjt == NT - 1:
                    emit_finish_pair(pb, pg)
            prev = (b, g, jt, expts)
        pb, pg, pjt, pexp = prev
        emit_attnv(pb, pg, pjt, pexp)
        emit_finish_pair(pb, pg)
        while q:
            q.pop(0)[1]()

        # ---- tail: yproj(1) ----
        for r in range(NT):
            u_yp(1, r)

    nc.compile()
    return nc


_NC = None


def _get_program():
    global _NC
    if _NC is None:
        _NC = build_program()
    return _NC


def make_in_maps(x, w_qkv, w_out, b_out, scale):
    x = np.ascontiguousarray(np.asarray(x, dtype=np.float32))
    w_qkv = np.ascontiguousarray(np.asarray(w_qkv, dtype=np.float32))
    w_out = np.ascontiguousarray(np.asarray(w_out, dtype=np.float32))
    b_out = np.ascontiguousarray(np.asarray(b_out, dtype=np.float32))
    scale = np.ascontiguousarray(np.asarray(scale, dtype=np.float32))
    return [
        {
            "x": x[c * BPC:(c + 1) * BPC],
            "w_qkv": w_qkv,
            "w_out": w_out,
            "b_out": b_out,
            "scale": scale,
        }
        for c in range(N_CORES)
    ]


def kernel(x, w_qkv, w_out, b_out, scale):
    nc = _get_program()
    in_maps = make_in_maps(x, w_qkv, w_out, b_out, scale)
    res = run_bass_kernel_spmd(nc, in_maps, core_ids=list(range(N_CORES)))
    return np.concatenate([res.results[c]["y"] for c in range(N_CORES)], axis=0)


if __name__ == "__main__":
    rng = np.random.default_rng(0)
    inputs = {
        "x": rng.standard_normal((B, N, D), dtype=np.float32),
        "w_qkv": rng.standard_normal((D, 3 * D), dtype=np.float32) * 0.03,
        "w_out": rng.standard_normal((D, D), dtype=np.float32) * 0.04,
        "b_out": np.zeros(D, dtype=np.float32),
        "scale": np.full(HEADS, DH ** -0.5, dtype=np.float32),
    }
    out = kernel(**inputs)
    print("kernel output", out.shape, out.dtype)
